# revision 1
# baseline (speedup 1.0000x reference)
"""Dice-coefficient-mean kernel v2 for Trainium2 (8 NeuronCores, SPMD).

Sharding: data-parallel over batch - core b processes batch b
(128^3 = 2,097,152 elements per tensor, laid out [128, 16384]).

Needed per core, per label l (42 numbers):
  inter[l] = #(s1==l & s2==l), c1[l] = #(s1==l), c2[l] = #(s2==l)

Measured engine rates (HW):
  DVE tensor_scalar mask (fp16, 4x mode): ~(FD/4+150)/0.96GHz
  DVE tensor_tensor (fp16, 2x): (FD/2+151)/0.96
  ScalarE ACTIVATE (1x, dtype-independent): (FD+352)/1.2
  PE ones-matmul reduce N=512: 235ns untiled, 59ns with 4-way
    column tiling (tile_position=(0,32t) runs 4 tiles concurrently)

Plan: per super-chunk (8192 cols, 2 per core):
  s1x = 16*s1 (TS mult, exact fp16); pair = s1x + s2 (TT add, exact)
  28 DVE mask stats: eq(pair==17l) l=0..13; c1 cums (s1x<=16T+8)
    T=0..11; c2 cums (s2<=T+.5) T=10,11. Masks reduced by 4-way
    column-tiled PE ones-matmuls, 4 stats per PSUM bank at
    partitions 0/32/64/96, accumulated across BOTH super-chunks
    (8 groups = 8 banks), then one [1,512] TR per stat row.
  2 PE moment chains (no mask): sum(s1), sum(s2) raw values.
  10 ACT Sign stats: c2 cums T=0..9, Sign(s2-(T+.5)) with accum.
Host decode in float64: diffs of cums; tails c[12],c[13] solved from
(N - cum11, moment). All counts are exact integers in fp32.
"""

import numpy as np

NUM_LABELS = 14
EPS = float(np.finfo(float).eps)
B = 8
P = 128
FREE = 16384
SC = 2
SCF = FREE // SC  # 8192
MH = 4096         # mask tile width (half super-chunk)
MM_N = 512
N_ACT = 9                # c2 cums T=0..8 on ScalarE
N_C1 = 12                # c1 cums T=0..11 on DVE
C2_DVE = [9, 10, 11]     # c2 cums on DVE

_CACHE = {}


NBANK = 8


def _jobs():
    """Flat job list. Job j accumulates in PSUM bank j%8 at quadrant
    position 32*((j%8 + j//8) % 4).

    Concurrent PE column-tile chains need BOTH distinct tile positions
    AND distinct PSUM banks (same-bank writes serialize the PE), so
    consecutive jobs rotate through banks with staggered positions.
    s2-only jobs lead so DVE/PE start before s1 / the pair prep land.
    """
    jobs = [("s1h", "le", t + 0.5) for t in range(N_C1)]
    jobs += [("s1h", "mom", 0.0)]
    jobs += [("pair", "eq", 17.0 * l) for l in range(NUM_LABELS)]
    jobs += [("s2h", "le", t + 0.5) for t in C2_DVE]
    jobs += [("s2h", "mom", 0.0)]
    return jobs


def _slot(j):
    bank = j % NBANK
    pos = 32 * ((bank + j // NBANK) % 4)
    return bank, pos


def _build():
    from concourse import bacc, mybir, tile

    op = mybir.AluOpType
    jobs = _jobs()

    nc = bacc.Bacc("TRN2", target_bir_lowering=False)
    s1 = nc.dram_tensor("s1", [P, FREE], mybir.dt.float32, kind="ExternalInput")
    s2 = nc.dram_tensor("s2", [P, FREE], mybir.dt.float32, kind="ExternalInput")
    out_p = nc.dram_tensor(
        "stats_pe", [P, NBANK], mybir.dt.float32, kind="ExternalOutput"
    )
    out_a = nc.dram_tensor(
        "stats_act", [P, N_ACT * SC], mybir.dt.float32, kind="ExternalOutput"
    )

    with tile.TileContext(nc) as tc:
        with (
            tc.tile_pool(name="data", bufs=2) as dpool,
            tc.tile_pool(name="mask", bufs=4) as maskp,
            tc.tile_pool(name="aux", bufs=1) as aux,
            tc.tile_pool(name="psum", bufs=1, space="PSUM") as psum,
        ):
            stats_p = aux.tile([P, NBANK], mybir.dt.float32)
            stats_a = aux.tile([P, N_ACT * SC], mybir.dt.float32)
            junk = aux.tile([P, SCF], mybir.dt.float8e4)
            ones = aux.tile([P, 1], mybir.dt.float16)
            biases = aux.tile([P, N_ACT], mybir.dt.float32)
            nc.vector.memset(ones[:], 1.0)
            nc.vector.memset(stats_p[:], 0.0)
            for i in range(N_ACT):
                nc.vector.memset(biases[:, i:i + 1], -(i + 0.5))
            accs = [
                psum.tile([P, MM_N], mybir.dt.float32, tag=f"acc{g}",
                          name=f"acc{g}")
                for g in range(NBANK)
            ]

            dummy = aux.tile([P, 1024], mybir.dt.float16)
            warm = aux.tile([P, 1], mybir.dt.float16)
            nc.vector.memset(dummy[:], 0.0)
            nc.scalar.activation(
                out=warm[:], in_=ones[:],
                func=mybir.ActivationFunctionType.Sign,
                bias=biases[:, 0:1], scale=1.0,
            )
            for sc in range(SC):
                s1h = dpool.tile([P, SCF], mybir.dt.float16, tag="s1h")
                s2h = dpool.tile([P, SCF], mybir.dt.float16, tag="s2h")
                s1x = dpool.tile([P, SCF], mybir.dt.float16, tag="s1x")
                pair = dpool.tile([P, SCF], mybir.dt.float16, tag="pair")
                lo = sc * SCF
                if sc == 1:
                    # gpsimd stall (WAW with the s1-sc1 DMA) so sc1
                    # transfers do not steal DMA bandwidth from sc0
                    nc.gpsimd.tensor_scalar(
                        out=s1h[:, 0:960], in0=dummy[:, 0:960],
                        scalar1=0.0, scalar2=None, op0=op.mult,
                    )
                # SWDGE casts f32->fp16 inline with the HBM load.
                # s1 first: it feeds the DVE mask pipeline (the binding
                # engine); s2 (ScalarE work) is delayed behind a gpsimd
                # stall so s1 gets full DMA bandwidth
                nc.gpsimd.dma_start(out=s1h[:], in_=s1[:, lo:lo + SCF])
                if sc == 0:
                    nc.gpsimd.tensor_scalar(
                        out=s2h[:, 0:256], in0=dummy[:, 0:256],
                        scalar1=0.0, scalar2=None, op0=op.mult,
                    )
                nc.gpsimd.dma_start(out=s2h[:], in_=s2[:, lo:lo + SCF])
                # ACT path: Sign stats on s2h (whole super-chunk per instr)
                for i in range(N_ACT):
                    nc.scalar.activation(
                        out=junk[:], in_=s2h[:],
                        func=mybir.ActivationFunctionType.Sign,
                        bias=biases[:, i:i + 1], scale=1.0,
                        accum_out=stats_a[:, sc * N_ACT + i:sc * N_ACT + i + 1],
                    )
                srcs = {"s1x": None, "s2h": s2h, "s1h": s1h}
                nmm = SCF // MM_N       # 16 N=512 matmuls per job per sc

                def emit_mm(acc, pos, rhs, k):
                    nc.tensor.matmul(
                        acc[pos:pos + 1, :],
                        ones[:],
                        rhs[:, k * MM_N:(k + 1) * MM_N],
                        start=(sc == 0 and k == 0),
                        stop=(sc == SC - 1 and k == nmm - 1),
                        tile_position=(0, pos),
                    )

                # Software-pipelined PE emission, depth ~3: each new mask
                # adds a 16-matmul stream; each slot emits 16 matmuls
                # round-robin one-at-a-time across live streams so
                # consecutive PE instructions hit different column
                # tiles/banks (same-tile LDWEIGHTS+MATMUL serialize at
                # ~300ns; interleaving streams them concurrently).
                streams = []  # [acc, pos, rhs, next_k]

                def pump(tokens, min_live=2):
                    # strict 1-each round-robin across live streams so
                    # consecutive PE instructions never target the same
                    # column tile (which would serialize); hold back
                    # unless >=min_live streams can alternate
                    while tokens > 0 and len(streams) >= min_live:
                        for st in list(streams):
                            if tokens <= 0:
                                break
                            emit_mm(st[0], st[1], st[2], st[3])
                            st[3] += 1
                            tokens -= 1
                            if st[3] >= nmm:
                                streams.remove(st)

                for j, (src, kind, const) in enumerate(jobs):
                    if j == 0:
                        # s1-only prep first; c1 masks keep DVE busy
                        # while s2 is still in flight
                        nc.vector.tensor_scalar(
                            out=s1x[:], in0=s1h[:], scalar1=16.0,
                            scalar2=None, op0=op.mult,
                        )
                    if j == N_C1 + 1:
                        # pair prep once s2h has landed
                        nc.vector.tensor_tensor(
                            out=pair[:], in0=s1x[:], in1=s2h[:], op=op.add,
                        )
                        srcs["pair"] = pair
                    bank, pos = _slot(j)
                    acc = accs[bank]
                    if kind == "mom":
                        streams.append([acc, pos, srcs[src], 0])
                    else:
                        mask = maskp.tile([P, SCF], mybir.dt.float16,
                                          tag="mask")
                        nc.vector.tensor_scalar(
                            out=mask[:],
                            in0=srcs[src][:],
                            scalar1=const, scalar2=None,
                            op0=(op.is_equal if kind == "eq"
                                 else op.is_le),
                        )
                        streams.append([acc, pos, mask, 0])
                    pump(nmm)
                while streams:
                    pump(nmm, min_live=1)
            for bank in range(NBANK):
                nc.vector.tensor_reduce(
                    out=stats_p[:, bank:bank + 1],
                    in_=accs[bank][:],
                    axis=mybir.AxisListType.X,
                    op=op.add,
                )
            nc.sync.dma_start(out=out_p[:], in_=stats_p[:])
            nc.sync.dma_start(out=out_a[:], in_=stats_a[:])
    nc.compile()
    return nc, jobs


def _get_built():
    if "k" not in _CACHE:
        _CACHE["k"] = _build()
    return _CACHE["k"]


LAST_EXEC_NS = None
LAST_RESULTS = None


def _decode(results, jobs):
    n_total = float(P * FREE)
    dice = np.zeros((B, NUM_LABELS), dtype=np.float64)
    for b in range(B):
        sp = np.asarray(results[b]["stats_pe"], dtype=np.float64)  # [P, NBANK]
        sa = np.asarray(results[b]["stats_act"], dtype=np.float64)
        vals = {}
        for j, (src, kind, const) in enumerate(jobs):
            bank, pos = _slot(j)
            vals[(src, kind, const)] = sp[pos, bank]
        inter = np.array([vals[("pair", "eq", 17.0 * l)]
                          for l in range(NUM_LABELS)])
        f1 = np.zeros(NUM_LABELS)
        for t in range(N_C1):
            f1[t] = vals[("s1h", "le", t + 0.5)]
        f2 = np.zeros(NUM_LABELS)
        for t in C2_DVE:
            f2[t] = vals[("s2h", "le", t + 0.5)]
        mom1 = vals[("s1h", "mom", 0.0)]
        mom2 = vals[("s2h", "mom", 0.0)]
        sa2 = sa.reshape(P, SC, N_ACT)
        for t in range(N_ACT):
            f2[t] = ((SCF - sa2[:, :, t]) / 2.0).sum()
        c1 = np.zeros(NUM_LABELS)
        c2 = np.zeros(NUM_LABELS)
        c1[0] = f1[0]
        c2[0] = f2[0]
        for t in range(1, 12):
            c1[t] = f1[t] - f1[t - 1]
            c2[t] = f2[t] - f2[t - 1]
        r1 = n_total - f1[11]
        m1r = mom1 - sum(v * c1[v] for v in range(12))
        c1[13] = m1r - 12.0 * r1
        c1[12] = r1 - c1[13]
        r2 = n_total - f2[11]
        m2r = mom2 - sum(v * c2[v] for v in range(12))
        c2[13] = m2r - 12.0 * r2
        c2[12] = r2 - c2[13]
        dice[b] = 2.0 * inter / (c1 + c2 + EPS)
    resv = dice.reshape(-1)
    total = resv.sum()
    nz = float((resv > 0).sum())
    mean = total / nz if nz > 0 else 0.0
    return np.float32(mean)


def _run(segment1, segment2, trace=False):
    global LAST_EXEC_NS, LAST_RESULTS
    from concourse.bass_utils import run_bass_kernel_spmd

    nc, jobs = _get_built()
    seg1 = np.ascontiguousarray(np.asarray(segment1, dtype=np.float32)).reshape(
        B, P, FREE
    )
    seg2 = np.ascontiguousarray(np.asarray(segment2, dtype=np.float32)).reshape(
        B, P, FREE
    )
    in_maps = [{"s1": seg1[b], "s2": seg2[b]} for b in range(B)]
    res = run_bass_kernel_spmd(nc, in_maps, core_ids=list(range(B)), trace=trace)
    LAST_EXEC_NS = res.exec_time_ns
    LAST_RESULTS = res
    return _decode(res.results, jobs)


def kernel(segment1, segment2):
    return _run(segment1, segment2, trace=False)


def benchmark(segment1, segment2):
    _run(segment1, segment2, trace=True)
    return LAST_EXEC_NS

